# revision 19
# baseline (speedup 1.0000x reference)
"""Trainium2 Bass kernel for ClusterAssignment (vq_codebook, t-distribution
soft assignment, ALPHA=1).

q[n,k] = num[n,k] / sum_k num[n,k],   num = 1/(1 + |z_n - c_k|^2)

Strategy (data-parallel over 8 NeuronCores, 8192 rows each):
  - host passes z^T per-core shard [128, 8192] (bf16) so the PE can contract
    over the feature dim d=128 without on-device transposes
  - u = 1 + |z|^2 + |c|^2 - 2 z.c built fully inside PSUM:
      * bias matmul (f32, contraction 9): block-diag-ones x zsq + ones x (1+csq)
      * 8 cross matmuls (bf16): lhsT = zT tile [128d,128n], rhs = -2 c^T [128d,64k]
    (zsq comes from exact f32 z; csq from the bf16-rounded centroids, so the
    PSUM value is exactly 1 + |z - c_bf16|^2 up to the z-side rounding)
  - num = 1/u via single custom-DVE reciprocal_approx_fast (~51 ULP)
  - row-normalize: grouped free-dim reduce (DVE) -> exact reciprocal of the
    [128,8] sums (DVE) -> broadcast tensor_tensor multiply (GPSIMD)
  - DMA split: z loads on qSP-HWDGE, q stores on qAct-HWDGE, consts on SWDGE
"""

import sys

if "/opt/trn_rl_repo" not in sys.path:
    sys.path.insert(0, "/opt/trn_rl_repo")

import ml_dtypes
import numpy as np

import concourse.bacc as bacc
import concourse.tile as tile
from concourse import mybir
from concourse.bass_interp import get_hw_module
from concourse.bass_utils import run_bass_kernel_spmd

N, K, D = 65536, 64, 128
NCORES = 8
NS = N // NCORES  # 8192 rows per core
NBANKS = 8  # pipeline stages per core (one full psum bank each)
BANK_N = NS // NBANKS  # 1024 rows per bank
TPB = BANK_N // 128  # 8 n-tiles of 128 rows per bank

MUL_ENGINE = "gpsimd"  # engine for the final broadcast multiply

_CACHE = {}


def _build_nc(iters=1):
    f32 = mybir.dt.float32
    bf16 = mybir.dt.bfloat16
    nc = bacc.Bacc(
        "TRN2",
        target_bir_lowering=False,
        debug=False,
        enable_asserts=False,
        num_devices=NCORES,
    )
    NB = 2 * TPB + 2  # bias matmul contraction: hi/lo zsq blocks + 2 ones rows
    zT = nc.dram_tensor("zT", [D, NS], bf16, kind="ExternalInput").ap()
    cTm2 = nc.dram_tensor("cTm2", [D, K], bf16, kind="ExternalInput").ap()
    blhs = nc.dram_tensor("blhs", [NB, NBANKS * 128], bf16, kind="ExternalInput").ap()
    brhs = nc.dram_tensor("brhs", [NB, TPB * K], bf16, kind="ExternalInput").ap()
    q = nc.dram_tensor("q", [NS, K], f32, kind="ExternalOutput").ap()

    # DRAM view of q matching the SBUF bank layout. Row n = b*1024 + p*8 + t
    # (p-major within a bank) so each partition's 512 floats are contiguous
    # in DRAM -> full-rate 2KB-chunk store DMA.
    q_banked = q.rearrange("(b p t) k -> b p (t k)", p=128, t=TPB)

    with tile.TileContext(nc) as tc:
        with (
            tc.tile_pool(name="const", bufs=1) as const_pool,
            tc.tile_pool(name="zin", bufs=6) as zin_pool,
            tc.tile_pool(name="work", bufs=4) as work_pool,
            tc.tile_pool(name="small", bufs=4) as small_pool,
            tc.tile_pool(name="psum", bufs=4, space="PSUM") as psum_pool,
        ):
            c_sb = const_pool.tile([D, K], bf16)
            nc.scalar.dma_start(c_sb[:], cTm2[:])
            blhs_sb = const_pool.tile([NB, NBANKS * 128], bf16)
            nc.gpsimd.dma_start(blhs_sb[:], blhs[:])
            brhs_sb = const_pool.tile([NB, TPB * K], bf16)
            nc.gpsimd.dma_start(brhs_sb[:], brhs[:])

            def body(b):
                zt = zin_pool.tile([D, BANK_N], bf16, tag="zt")
                nc.sync.dma_start(zt[:], zT[:, b * BANK_N : (b + 1) * BANK_N])

                ps = psum_pool.tile([128, TPB * K], f32, tag="ps")
                # -2 z.c per 128-row tile first (doesn't wait on the bias
                # consts), then the zsq/csq bias accumulated on top
                for t in range(TPB):
                    nc.tensor.matmul(
                        ps[:, t * K : (t + 1) * K],
                        zt[:, t * 128 : (t + 1) * 128],
                        c_sb[:],
                        start=(t == 0),  # start clears the whole bank's
                        stop=False,  # has_written bits, so only once
                    )
                nc.tensor.matmul(
                    ps[:],
                    blhs_sb[:, b * 128 : (b + 1) * 128],
                    brhs_sb[:],
                    start=False,
                    stop=True,
                )

                num = work_pool.tile([128, TPB * K], f32, tag="num")
                nc.vector.reciprocal_approx_fast(out=num[:], in_=ps[:])

                s = small_pool.tile([128, TPB], f32, tag="s")
                nc.vector.reduce_sum(
                    out=s[:],
                    in_=num[:].rearrange("p (t k) -> p t k", k=K),
                    axis=mybir.AxisListType.X,
                )
                sinv = small_pool.tile([128, TPB], f32, tag="sinv")
                nc.vector.reciprocal(out=sinv[:], in_=s[:])

                qt = work_pool.tile([128, TPB * K], f32, tag="qt")
                mul_eng = getattr(nc, MUL_ENGINE)
                mul_eng.tensor_mul(
                    qt[:].rearrange("p (t k) -> p t k", k=K),
                    num[:].rearrange("p (t k) -> p t k", k=K),
                    sinv[:].broadcast_to([128, TPB, K]),
                )
                nc.scalar.dma_start(q_banked[b], qt[:])

            if iters == 1:
                for b in range(NBANKS):
                    body(b)
            else:
                # benchmark mode: repeat the whole pipeline on-device
                with tc.For_i(0, iters, 1):
                    for b in range(NBANKS):
                        body(b)

    nc.compile()
    nc.m = get_hw_module(nc.m)
    return nc


def _get_nc():
    if "nc" not in _CACHE:
        _CACHE["nc"] = _build_nc()
    return _CACHE["nc"]


def _hilo(x):
    """Split f64 values into bf16 hi + bf16 lo with hi+lo ~ x to ~16 bits."""
    hi = x.astype(ml_dtypes.bfloat16)
    lo = (x - hi.astype(np.float64)).astype(ml_dtypes.bfloat16)
    return hi, lo


def _host_prep(z, centroids):
    """Per-core input maps.

    SBUF column order within a bank is (t, p) but the row it holds is
    n = bank*1024 + p*8 + t (p-major) so the q store is DRAM-contiguous
    per partition.
    """
    z = np.asarray(z, dtype=np.float32)
    c = np.asarray(centroids, dtype=np.float32)
    NB = 2 * TPB + 2

    c_bf = (-2.0 * c.T).astype(ml_dtypes.bfloat16)  # [D, K]
    # csq must match the centroids the PE actually sees (bf16-rounded)
    c_eff = (c.T.astype(ml_dtypes.bfloat16)).astype(np.float64).T  # [K, D]
    csq1 = 1.0 + (c_eff**2).sum(axis=1)  # [K] f64
    csq1_hi, csq1_lo = _hilo(csq1)

    brhs = np.zeros((NB, TPB * K), dtype=ml_dtypes.bfloat16)
    for t in range(TPB):
        brhs[t, t * K : (t + 1) * K] = 1.0
        brhs[TPB + t, t * K : (t + 1) * K] = 1.0
    brhs[2 * TPB, :] = np.tile(csq1_hi, TPB)
    brhs[2 * TPB + 1, :] = np.tile(csq1_lo, TPB)

    in_maps = []
    for i in range(NCORES):
        zs = z[i * NS : (i + 1) * NS]  # [NS, D]
        # column (b*1024 + t*128 + p) of zT holds row b*1024 + p*8 + t
        z_perm = (
            zs.reshape(NBANKS, 128, TPB, D).transpose(0, 2, 1, 3).reshape(NS, D)
        )
        zT = np.ascontiguousarray(z_perm.T).astype(ml_dtypes.bfloat16)  # [D, NS]

        zsq = (zs.astype(np.float64) ** 2).sum(axis=1)  # [NS] f64
        zsq_hi, zsq_lo = _hilo(zsq)
        # blhs[t, b*128+p] = zsq[b*1024 + p*8 + t] (hi rows then lo rows)
        blhs = np.empty((NB, NBANKS * 128), dtype=ml_dtypes.bfloat16)
        blhs[:TPB] = zsq_hi.reshape(NBANKS, 128, TPB).transpose(2, 0, 1).reshape(TPB, -1)
        blhs[TPB : 2 * TPB] = (
            zsq_lo.reshape(NBANKS, 128, TPB).transpose(2, 0, 1).reshape(TPB, -1)
        )
        blhs[2 * TPB :] = 1.0
        in_maps.append({"zT": zT, "cTm2": c_bf, "blhs": blhs, "brhs": brhs})
    return in_maps


def kernel(z, centroids):
    nc = _get_nc()
    in_maps = _host_prep(z, centroids)
    res = run_bass_kernel_spmd(nc, in_maps, list(range(NCORES)))
    out = np.concatenate([res.results[i]["q"] for i in range(NCORES)], axis=0)
    return out.astype(np.float32)
